# revision 2
# baseline (speedup 1.0000x reference)
"""HQQ 4-bit quantized linear on 8 Trainium2 NeuronCores (Bass/Tile).

out[4096, 11008] = x[4096, 4096] @ dequant(W_q, scale, zero).T + bias

Core c owns output columns [c*1376, (c+1)*1376) (column-parallel, x
replicated): o = g_row*172 + j, group g = j*4096 + i; core c holds
g_rows 8*(c%4)..8*(c%4)+8 of the hi (c<4) / lo (c>=4) nibble plane.

Host-side marshalling (bit/layout repack only; dequant + matmul run on
device): x.T fp16, nibbles unpacked to ONE NIBBLE PER BYTE (u8) in
[i, o] layout (5.6MB/core vs 11.3MB as fp16 -- the phase-1 DMA stream
is what used to stall the first token-pair), scale/zero transposed+
interleaved fp16, bias row replicated.

Device per core (PE runs zero transposes):
  phase 1: szt is DMA'd up-front into a resident [128, NK, 2, J] tile
      (8 chunked DMAs, no ring pacing); nib u8 streams through a
      6-deep kp ring on the ScalarE HWDGE queue.  Per k-block:
      2-of-3 k's are ACT-converted u8->fp16 then DVE fp16-sub
      (2x mode), 1-of-3 subs run directly on GPSIMD from u8; the
      scale mul runs in place on WT (DVE 2x), with GPSIMD-k muls
      deferred 2 k-blocks so they never stall the strict-FIFO DVE
      queue.  Engine budget/k-window: DVE ~32us, ACT ~27us, GPSIMD
      ~31us vs the 36.7us pair-0 sweep; DMA 8.4MB vs ~13MB of
      window supply.  No warm-up matmuls: real matmuls start right
      after the engine preamble and warm the HAM clock themselves
      (supply runs ~2.4x faster than the cold-clock k consumption).
  phase 2: token tiles in pairs, k-outer PSUM accumulation (6 banks
      live, rotating through all 8 to avoid WAR bubbles); drain = DVE
      bias-add; x prefetched one 2MB pair ahead, first pair issued
      before any dequant traffic.  The last pair runs o-split-outer /
      k-inner on the two banks left free by pair 14 (then the
      just-drained ones), inits PSUM via K=1 bias matmuls, and drains
      each split (ACT copy for u0, DVE for u1) while later splits still
      compute -- the post-matmul tail is only the final 352-col split.
"""

import numpy as np
from contextlib import ExitStack

import concourse.bacc as bacc
import concourse.bass as bass
import concourse.mybir as mybir
import concourse.tile as tile
from concourse.bass_utils import run_bass_kernel_spmd

dt = mybir.dt

TOKENS, IN_F, OUT_F, GS = 4096, 4096, 11008, 64
G = OUT_F * IN_F // GS            # 704512 quantization groups
J = G // IN_F                     # 172 groups per (g_row, i) plane
NCORES = 8
RPC = GS // NCORES                # 8 g_rows per core
O_C = RPC * J                     # 1376 output cols per core
NK = IN_F // 128                  # 32 contraction blocks
TQ = 256                          # tokens per x-buffer chunk (1 pair)
NQ = TOKENS // TQ                 # 16 pairs
O_SPLITS = ((0, 512), (512, 512), (1024, 352))   # psum o-tiles (1 bank each)

_CACHE = {}


def _build():
    nc = bacc.Bacc("TRN2", target_bir_lowering=False, debug=False,
                   num_devices=NCORES)

    xt_d = nc.dram_tensor("xt", [IN_F, TOKENS], dt.float16,
                          kind="ExternalInput")
    nibf_d = nc.dram_tensor("nibf", [IN_F, O_C], dt.uint8,
                            kind="ExternalInput")
    szt_d = nc.dram_tensor("szt", [IN_F, 2, J], dt.float16,
                           kind="ExternalInput")
    b_d = nc.dram_tensor("bias", [128, O_C], dt.float32,
                         kind="ExternalInput")
    o_d = nc.dram_tensor("out", [TOKENS, O_C], dt.float32,
                         kind="ExternalOutput")

    with ExitStack() as ctx:
        tc = ctx.enter_context(tile.TileContext(nc))
        const = ctx.enter_context(tc.tile_pool(name="const", bufs=1))
        ph1 = ctx.enter_context(tc.tile_pool(name="ph1", bufs=6))
        xpool = ctx.enter_context(tc.tile_pool(name="xpool", bufs=2))
        opool = ctx.enter_context(tc.tile_pool(name="opool", bufs=1))
        pacc = ctx.enter_context(
            tc.tile_pool(name="pacc", bufs=1, space=bass.MemorySpace.PSUM))

        biasrep = const.tile([128, O_C], dt.float32)

        # resident scale/zero: [i-part, k-block, {scale,zero}, j]
        szt = const.tile([128, NK, 2, J], dt.float16)

        # resident transposed dequantized weights: [i-part, k-block, r, j]
        WT = const.tile([128, NK, RPC, J], dt.float16)

        # x-pair prefetch on the SP (sync) DMA stream; issue the first two
        # before any dequant DMA so nothing head-blocks them.
        xbs = {}

        def fetch(q, chunks=1):
            xb = xpool.tile([128, NK, TQ], dt.float16, tag="xb",
                            name=f"xb{q % 2}")
            kc = NK // chunks
            src = xt_d[:, q * TQ:(q + 1) * TQ].rearrange(
                "(k p) t -> p k t", p=128)
            for c in range(chunks):
                nc.sync.dma_start(
                    xb[:, c * kc:(c + 1) * kc], src[:, c * kc:(c + 1) * kc])
            xbs[q] = xb

        fetch(0, chunks=4)
        fetch(1)
        nc.sync.dma_start(biasrep[:], b_d[:])
        biash = const.tile([1, O_C], dt.float16)
        nc.scalar.copy(biash[:], biasrep[0:1, :])
        ones = const.tile([1, 128], dt.float16)
        nc.vector.memset(ones[:], 1.0)

        # ---- phase 1: dequant (layout already [i, o]; no transposes) ----
        # All DMAs on the ScalarE HWDGE stream (keeps the SP queue's x/out
        # DMAs unblocked); szt chunks are unpaced, nib paces via its ring
        # but a blocked nib dma_start only delays that k's own convert.
        pend = []

        def _flush(k):
            nc.vector.tensor_mul(
                WT[:, k], WT[:, k],
                szt[:, k, 0].unsqueeze(1).broadcast_to((128, RPC, J)))

        for kp in range(NK // 2):
            if kp % 2 == 0:
                g = kp // 2          # szt chunk: 4 k-blocks = 512 i-rows
                nc.scalar.dma_start(
                    szt[:, 4 * g:4 * (g + 1)],
                    szt_d[g * 512:(g + 1) * 512].rearrange(
                        "(b p) z j -> p b z j", p=128))
            nib2 = ph1.tile([128, 2, RPC, J], dt.uint8, tag="nib2", bufs=6)
            if kp == 0:
                for b in range(2):
                    nc.scalar.dma_start(
                        nib2[:, b],
                        nibf_d[b * 128:(b + 1) * 128, :].rearrange(
                            "p (r j) -> p r j", r=RPC))
            else:
                nc.scalar.dma_start(
                    nib2[:], nibf_d[kp * 256:(kp + 1) * 256, :].rearrange(
                        "(b p) (r j) -> p b r j", p=128, r=RPC))
            for b in range(2):
                k = kp * 2 + b
                zero_ap = szt[:, k, 1].unsqueeze(1).broadcast_to(
                    (128, RPC, J))
                if k >= 2 and (k % 3) == 2:
                    # GPSIMD reads the u8 nibbles directly; its mul is
                    # deferred two k-blocks on the DVE queue.
                    nc.gpsimd.tensor_sub(WT[:, k], nib2[:, b], zero_ap)
                    pend.append(k)
                else:
                    conv = ph1.tile([128, RPC, J], dt.float16, tag="conv",
                                    bufs=4)
                    nc.scalar.copy(conv[:], nib2[:, b])
                    nc.vector.tensor_sub(WT[:, k], conv[:], zero_ap)
                    _flush(k)
                while pend and pend[0] <= k - 2:
                    _flush(pend.pop(0))
        while pend:
            _flush(pend.pop(0))

        # ---- phase 2: stream xT, pair-wise k-outer matmul ----
        # accumulator banks: 512 fp32 = 1 PSUM bank each; rotate 6-of-8
        # per pair so the next pair starts on just-freed banks.
        for q in range(NQ):
            if 2 <= q + 1 < NQ:
                fetch(q + 1)
            xb = xbs.pop(q)
            if q < NQ - 1:
                accs = [[pacc.tile([128, 512], dt.float32,
                                   tag=f"a{(q * 6 + u * 3 + p) % 8}",
                                   name=f"acc{(q * 6 + u * 3 + p) % 8}"
                                   )[:, 0:on]
                         for p, (ob, on) in enumerate(O_SPLITS)]
                        for u in range(2)]
                for k in range(NK):
                    wk = WT[:, k].rearrange("p r j -> p (r j)")
                    for u in range(2):
                        lhsT = xb[:, k, u * 128:(u + 1) * 128]
                        for p, (ob, on) in enumerate(O_SPLITS):
                            nc.tensor.matmul(
                                accs[u][p][:], lhsT, wk[:, ob:ob + on],
                                start=(k == 0), stop=(k == NK - 1))
                for u in range(2):
                    t = q * 2 + u
                    for p, (ob, on) in enumerate(O_SPLITS):
                        ot = opool.tile([128, on], dt.float32,
                                        tag=f"o{u}{p}", name=f"ot{u}{p}")
                        nc.vector.tensor_add(
                            ot[:], accs[u][p][:], biasrep[:, ob:ob + on])
                        nc.sync.dma_start(
                            o_d[t * 128:(t + 1) * 128, ob:ob + on], ot[:])
            else:
                # last pair: o-split-outer / k-inner so each split drains
                # while the next computes; start on the banks pair 14
                # leaves free (a2, a3), then its just-drained ones.
                tags = ["a2", "a3", "a4", "a5", "a6", "a7"]
                for p, (ob, on) in enumerate(O_SPLITS):
                    for u in range(2):
                        acc = pacc.tile([128, 512], dt.float32,
                                        tag=tags[p * 2 + u],
                                        name=f"lacc{p}{u}")[:, 0:on]
                        nc.tensor.matmul(
                            acc[:], ones[0:1, :], biash[0:1, ob:ob + on],
                            start=True, stop=False)
                        for k in range(NK):
                            wk = WT[:, k].rearrange("p r j -> p (r j)")
                            lhsT = xb[:, k, u * 128:(u + 1) * 128]
                            nc.tensor.matmul(
                                acc[:], lhsT, wk[:, ob:ob + on],
                                start=False, stop=(k == NK - 1))
                        t = q * 2 + u
                        ot = opool.tile([128, on], dt.float32,
                                        tag=f"o{u}{p}", name=f"ot{u}{p}")
                        if u == 0:
                            nc.scalar.copy(ot[:], acc[:])
                        else:
                            nc.vector.tensor_copy(ot[:], acc[:])
                        nc.sync.dma_start(
                            o_d[t * 128:(t + 1) * 128, ob:ob + on], ot[:])

    nc.compile()
    return nc


def get_nc():
    if "nc" not in _CACHE:
        _CACHE["nc"] = _build()
    return _CACHE["nc"]


def make_in_maps(x, W_q, scale, zero, bias):
    x = np.ascontiguousarray(x, dtype=np.float32)
    xt = np.ascontiguousarray(x.T).astype(np.float16)
    st = np.asarray(scale, dtype=np.float32).reshape(J, IN_F).T.astype(
        np.float16)
    zt = np.asarray(zero, dtype=np.float32).reshape(J, IN_F).T.astype(
        np.float16)
    szt = np.ascontiguousarray(np.stack([st, zt], axis=1))   # [IN_F, 2, J]
    bias = np.ascontiguousarray(bias, dtype=np.float32)
    Wb = np.asarray(W_q, dtype=np.int32).astype(np.uint8)   # [32, G]
    in_maps = [None] * NCORES
    for cg in range(4):
        slab = Wb[RPC * cg:RPC * (cg + 1)]                  # [8, G]
        for half, c in ((slab >> 4, cg), (slab & 15, cg + 4)):
            nib = np.ascontiguousarray(
                half.reshape(RPC, J, IN_F).transpose(2, 0, 1)
            ).reshape(IN_F, O_C)
            in_maps[c] = {
                "xt": xt,
                "nibf": nib,                                # u8, one nib/byte
                "szt": szt,
                "bias": np.ascontiguousarray(np.broadcast_to(
                    bias[c * O_C:(c + 1) * O_C], (128, O_C))),
            }
    return in_maps


def kernel(x, W_q, scale, zero, bias):
    nc = get_nc()
    in_maps = make_in_maps(x, W_q, scale, zero, bias)
    res = run_bass_kernel_spmd(nc, in_maps, list(range(NCORES)))
    return np.concatenate(
        [res.results[c]["out"] for c in range(NCORES)], axis=1)


# revision 3
# speedup vs baseline: 1.1740x; 1.1740x over previous
"""HQQ 4-bit quantized linear on 8 Trainium2 NeuronCores (Bass/Tile).

out[4096, 11008] = x[4096, 4096] @ dequant(W_q, scale, zero).T + bias

Core c owns output columns [c*1376, (c+1)*1376) (column-parallel, x
replicated): o = g_row*172 + j, group g = j*4096 + i; core c holds
g_rows 8*(c%4)..8*(c%4)+8 of the hi (c<4) / lo (c>=4) nibble plane.

Host-side marshalling (bit/layout repack only; dequant + matmul run on
device): x.T fp16, nibbles unpacked to ONE NIBBLE PER BYTE (u8) in
[i, o] layout (5.6MB/core vs 11.3MB as fp16 -- the phase-1 DMA stream
is what used to stall the first token-pair), scale/zero transposed+
interleaved fp16, bias row replicated.

Device per core (PE runs zero transposes):
  phase 1: szt is DMA'd up-front into a resident [128, NK, 2, J] tile
      (8 chunked DMAs, no ring pacing); nib u8 streams through a
      6-deep kp ring on the ScalarE HWDGE queue.  Per k-block:
      2-of-3 k's are ACT-converted u8->fp16 then DVE fp16-sub
      (2x mode), 1-of-3 subs run directly on GPSIMD from u8; the
      scale mul runs in place on WT (DVE 2x), with GPSIMD-k muls
      deferred 2 k-blocks so they never stall the strict-FIFO DVE
      queue.  Engine budget/k-window: DVE ~32us, ACT ~27us, GPSIMD
      ~31us vs the 36.7us pair-0 sweep; DMA 8.4MB vs ~13MB of
      window supply.  No warm-up matmuls: real matmuls start right
      after the engine preamble and warm the HAM clock themselves
      (supply runs ~2.4x faster than the cold-clock k consumption).
  phase 2: token tiles in pairs, k-outer PSUM accumulation (6 banks
      live, rotating through all 8 to avoid WAR bubbles); drain = DVE
      bias-add; x prefetched one 2MB pair ahead, first pair issued
      before any dequant traffic.  The last pair runs o-split-outer /
      k-inner on the two banks left free by pair 14 (then the
      just-drained ones), inits PSUM via K=1 bias matmuls, and drains
      each split (ACT copy for u0, DVE for u1) while later splits still
      compute -- the post-matmul tail is only the final 352-col split.
"""

import numpy as np
from contextlib import ExitStack

import concourse.bacc as bacc
import concourse.bass as bass
import concourse.mybir as mybir
import concourse.tile as tile
from concourse.bass_utils import run_bass_kernel_spmd

dt = mybir.dt

TOKENS, IN_F, OUT_F, GS = 4096, 4096, 11008, 64
G = OUT_F * IN_F // GS            # 704512 quantization groups
J = G // IN_F                     # 172 groups per (g_row, i) plane
NCORES = 8
RPC = GS // NCORES                # 8 g_rows per core
O_C = RPC * J                     # 1376 output cols per core
NK = IN_F // 128                  # 32 contraction blocks
TQ = 256                          # tokens per x-buffer chunk (1 pair)
NQ = TOKENS // TQ                 # 16 pairs
O_SPLITS = ((0, 512), (512, 512), (1024, 352))   # psum o-tiles (1 bank each)

_CACHE = {}


def _build():
    nc = bacc.Bacc("TRN2", target_bir_lowering=False, debug=False,
                   num_devices=NCORES)

    xt_d = nc.dram_tensor("xt", [IN_F, TOKENS], dt.float16,
                          kind="ExternalInput")
    nibf_d = nc.dram_tensor("nibf", [IN_F, O_C], dt.uint8,
                            kind="ExternalInput")
    szt_d = nc.dram_tensor("szt", [IN_F, 2, J], dt.float16,
                           kind="ExternalInput")
    b_d = nc.dram_tensor("bias", [128, O_C], dt.float32,
                         kind="ExternalInput")
    o_d = nc.dram_tensor("out", [TOKENS, O_C], dt.float32,
                         kind="ExternalOutput")

    with ExitStack() as ctx:
        tc = ctx.enter_context(tile.TileContext(nc))
        const = ctx.enter_context(tc.tile_pool(name="const", bufs=1))
        ph1 = ctx.enter_context(tc.tile_pool(name="ph1", bufs=6))
        xpool = ctx.enter_context(tc.tile_pool(name="xpool", bufs=2))
        opool = ctx.enter_context(tc.tile_pool(name="opool", bufs=1))
        pacc = ctx.enter_context(
            tc.tile_pool(name="pacc", bufs=1, space=bass.MemorySpace.PSUM))

        biasrep = const.tile([128, O_C], dt.float32)

        # resident scale/zero: [i-part, k-block, {scale,zero}, j]
        szt = const.tile([128, NK, 2, J], dt.float16)

        # resident transposed dequantized weights: [i-part, k-block, r, j]
        WT = const.tile([128, NK, RPC, J], dt.float16)

        # x-pair prefetch on the SP (sync) DMA stream; issue the first two
        # before any dequant DMA so nothing head-blocks them.
        xbs = {}

        def fetch(q, chunks=1):
            xb = xpool.tile([128, NK, TQ], dt.float16, tag="xb",
                            name=f"xb{q % 2}")
            kc = NK // chunks
            src = xt_d[:, q * TQ:(q + 1) * TQ].rearrange(
                "(k p) t -> p k t", p=128)
            for c in range(chunks):
                nc.sync.dma_start(
                    xb[:, c * kc:(c + 1) * kc], src[:, c * kc:(c + 1) * kc])
            xbs[q] = xb

        # Interleave the szt chunk DMAs into the x/bias issue stream on the
        # SP queue (a dma_start costs ~0.9us of issuing-queue occupancy, so
        # placement matters): xb0 chunk 0 and szt chunk 0 first.
        xb0 = xpool.tile([128, NK, TQ], dt.float16, tag="xb", name="xb0")
        x0src = xt_d[:, 0:TQ].rearrange("(k p) t -> p k t", p=128)
        sztsrc = szt_d[:, :].rearrange("(g b p) z j -> p g b z j",
                                       p=128, b=4)

        def szt_chunk(g):
            nc.sync.dma_start(szt[:, 4 * g:4 * (g + 1)], sztsrc[:, g])

        nc.sync.dma_start(xb0[:, 0:8], x0src[:, 0:8])
        szt_chunk(0)
        for c in range(1, 4):
            nc.sync.dma_start(xb0[:, c * 8:(c + 1) * 8], x0src[:, c * 8:(c + 1) * 8])
        xbs[0] = xb0
        fetch(1)
        for g in range(1, 8):
            szt_chunk(g)
        nc.sync.dma_start(biasrep[:], b_d[:])
        ones = const.tile([1, 128], dt.float16)
        nc.vector.memset(ones[:], 1.0)

        # ---- phase 1: dequant (layout already [i, o]; no transposes) ----
        # nib u8 DMAs issue from the ScalarE queue in [1, 3, 4x7] k-block
        # batches (few, large issues; k0 alone so the first sub starts
        # early).  Subs: k%3==2 on GPSIMD (u8 direct, mul deferred 2
        # k-blocks), k%3==1 via ACT u8->fp16 convert + DVE 2x fp16 sub,
        # k%3==0 DVE u8 sub (1x).  Muls run in place on WT (DVE 2x).
        nibsrc = nibf_d[:, :].rearrange("(k p) (r j) -> p k r j",
                                        p=128, r=RPC)
        nibt = {}

        def nib_issue(k0, nk):
            t = ph1.tile([128, nk, RPC, J], dt.uint8, tag=f"nib{nk}",
                         bufs=(3 if nk == 4 else 1))
            nc.scalar.dma_start(t[:], nibsrc[:, k0:k0 + nk])
            for i in range(nk):
                nibt[k0 + i] = t[:, i]

        nib_issue(0, 1)
        nib_issue(1, 3)
        nib_issue(4, 4)

        pend = []

        def _flush(k):
            nc.vector.tensor_mul(
                WT[:, k], WT[:, k],
                szt[:, k, 0].unsqueeze(1).broadcast_to((128, RPC, J)))

        for k in range(NK):
            if k % 4 == 0 and k + 8 < NK:
                nib_issue(k + 8, 4)
            zero_ap = szt[:, k, 1].unsqueeze(1).broadcast_to((128, RPC, J))
            if k >= 2 and (k % 3) == 2:
                nc.gpsimd.tensor_sub(WT[:, k], nibt[k], zero_ap)
                pend.append(k)
            else:
                if k % 3 == 1:
                    conv = ph1.tile([128, RPC, J], dt.float16, tag="conv",
                                    bufs=4)
                    nc.scalar.copy(conv[:], nibt[k])
                    nc.vector.tensor_sub(WT[:, k], conv[:], zero_ap)
                else:
                    nc.vector.tensor_sub(WT[:, k], nibt[k], zero_ap)
                _flush(k)
            while pend and pend[0] <= k - 2:
                _flush(pend.pop(0))
        while pend:
            _flush(pend.pop(0))

        # biash only feeds the last pair's PSUM init; emitting it here
        # keeps it from head-blocking the ACT queue's phase-1 work.
        biash = const.tile([1, O_C], dt.float16)
        nc.scalar.copy(biash[:], biasrep[0:1, :])

        # ---- phase 2: stream xT, pair-wise k-outer matmul ----
        # accumulator banks: 512 fp32 = 1 PSUM bank each; rotate 6-of-8
        # per pair so the next pair starts on just-freed banks.
        for q in range(NQ):
            if 2 <= q + 1 < NQ:
                fetch(q + 1)
            xb = xbs.pop(q)
            if q < NQ - 1:
                accs = [[pacc.tile([128, 512], dt.float32,
                                   tag=f"a{(q * 6 + u * 3 + p) % 8}",
                                   name=f"acc{(q * 6 + u * 3 + p) % 8}"
                                   )[:, 0:on]
                         for p, (ob, on) in enumerate(O_SPLITS)]
                        for u in range(2)]
                for k in range(NK):
                    wk = WT[:, k].rearrange("p r j -> p (r j)")
                    for u in range(2):
                        lhsT = xb[:, k, u * 128:(u + 1) * 128]
                        for p, (ob, on) in enumerate(O_SPLITS):
                            nc.tensor.matmul(
                                accs[u][p][:], lhsT, wk[:, ob:ob + on],
                                start=(k == 0), stop=(k == NK - 1))
                for u in range(2):
                    t = q * 2 + u
                    for p, (ob, on) in enumerate(O_SPLITS):
                        ot = opool.tile([128, on], dt.float32,
                                        tag=f"o{u}{p}", name=f"ot{u}{p}")
                        nc.vector.tensor_add(
                            ot[:], accs[u][p][:], biasrep[:, ob:ob + on])
                        nc.sync.dma_start(
                            o_d[t * 128:(t + 1) * 128, ob:ob + on], ot[:])
            else:
                # last pair: o-split-outer / k-inner so each split drains
                # while the next computes; start on the banks pair 14
                # leaves free (a2, a3), then its just-drained ones.
                tags = ["a2", "a3", "a4", "a5", "a6", "a7"]
                for p, (ob, on) in enumerate(O_SPLITS):
                    for u in range(2):
                        acc = pacc.tile([128, 512], dt.float32,
                                        tag=tags[p * 2 + u],
                                        name=f"lacc{p}{u}")[:, 0:on]
                        nc.tensor.matmul(
                            acc[:], ones[0:1, :], biash[0:1, ob:ob + on],
                            start=True, stop=False)
                        for k in range(NK):
                            wk = WT[:, k].rearrange("p r j -> p (r j)")
                            lhsT = xb[:, k, u * 128:(u + 1) * 128]
                            nc.tensor.matmul(
                                acc[:], lhsT, wk[:, ob:ob + on],
                                start=False, stop=(k == NK - 1))
                        t = q * 2 + u
                        ot = opool.tile([128, on], dt.float32,
                                        tag=f"o{u}{p}", name=f"ot{u}{p}")
                        if u == 0:
                            nc.scalar.copy(ot[:], acc[:])
                        else:
                            nc.vector.tensor_copy(ot[:], acc[:])
                        nc.sync.dma_start(
                            o_d[t * 128:(t + 1) * 128, ob:ob + on], ot[:])

    nc.compile()
    return nc


def get_nc():
    if "nc" not in _CACHE:
        _CACHE["nc"] = _build()
    return _CACHE["nc"]


def make_in_maps(x, W_q, scale, zero, bias):
    x = np.ascontiguousarray(x, dtype=np.float32)
    xt = np.ascontiguousarray(x.T).astype(np.float16)
    st = np.asarray(scale, dtype=np.float32).reshape(J, IN_F).T.astype(
        np.float16)
    zt = np.asarray(zero, dtype=np.float32).reshape(J, IN_F).T.astype(
        np.float16)
    szt = np.ascontiguousarray(np.stack([st, zt], axis=1))   # [IN_F, 2, J]
    bias = np.ascontiguousarray(bias, dtype=np.float32)
    Wb = np.asarray(W_q, dtype=np.int32).astype(np.uint8)   # [32, G]
    in_maps = [None] * NCORES
    for cg in range(4):
        slab = Wb[RPC * cg:RPC * (cg + 1)]                  # [8, G]
        for half, c in ((slab >> 4, cg), (slab & 15, cg + 4)):
            nib = np.ascontiguousarray(
                half.reshape(RPC, J, IN_F).transpose(2, 0, 1)
            ).reshape(IN_F, O_C)
            in_maps[c] = {
                "xt": xt,
                "nibf": nib,                                # u8, one nib/byte
                "szt": szt,
                "bias": np.ascontiguousarray(np.broadcast_to(
                    bias[c * O_C:(c + 1) * O_C], (128, O_C))),
            }
    return in_maps


def kernel(x, W_q, scale, zero, bias):
    nc = get_nc()
    in_maps = make_in_maps(x, W_q, scale, zero, bias)
    res = run_bass_kernel_spmd(nc, in_maps, list(range(NCORES)))
    return np.concatenate(
        [res.results[c]["out"] for c in range(NCORES)], axis=1)


# revision 4
# speedup vs baseline: 1.1949x; 1.0178x over previous
"""HQQ 4-bit quantized linear on 8 Trainium2 NeuronCores (Bass/Tile).

out[4096, 11008] = x[4096, 4096] @ dequant(W_q, scale, zero).T + bias

Core c owns output columns [c*1376, (c+1)*1376) (column-parallel, x
replicated): o = g_row*172 + j, group g = j*4096 + i; core c holds
g_rows 8*(c%4)..8*(c%4)+8 of the hi (c<4) / lo (c>=4) nibble plane.

Host-side marshalling (bit/layout repack only; dequant + matmul run on
device): x.T fp16, nibbles unpacked to ONE NIBBLE PER BYTE (u8) in
[i, o] layout (5.6MB/core vs 11.3MB as fp16 -- the phase-1 DMA stream
is what used to stall the first token-pair), scale/zero transposed+
interleaved fp16, bias row replicated.

Device per core (PE runs zero transposes):
  phase 1: szt is DMA'd up-front into a resident [128, NK, 2, J] tile
      (8 chunked DMAs, no ring pacing); nib u8 streams through a
      6-deep kp ring on the ScalarE HWDGE queue.  Per k-block:
      2-of-3 k's are ACT-converted u8->fp16 then DVE fp16-sub
      (2x mode), 1-of-3 subs run directly on GPSIMD from u8; the
      scale mul runs in place on WT (DVE 2x), with GPSIMD-k muls
      deferred 2 k-blocks so they never stall the strict-FIFO DVE
      queue.  Engine budget/k-window: DVE ~32us, ACT ~27us, GPSIMD
      ~31us vs the 36.7us pair-0 sweep; DMA 8.4MB vs ~13MB of
      window supply.  No warm-up matmuls: real matmuls start right
      after the engine preamble and warm the HAM clock themselves
      (supply runs ~2.4x faster than the cold-clock k consumption).
  phase 2: token tiles in pairs, k-outer PSUM accumulation (6 banks
      live, rotating through all 8 to avoid WAR bubbles); drain = DVE
      bias-add; x prefetched one 2MB pair ahead, first pair issued
      before any dequant traffic.  The last pair runs o-split-outer /
      k-inner on the two banks left free by pair 14 (then the
      just-drained ones), inits PSUM via K=1 bias matmuls, and drains
      each split (ACT copy for u0, DVE for u1) while later splits still
      compute -- the post-matmul tail is only the final 352-col split.
"""

import numpy as np
from contextlib import ExitStack

import concourse.bacc as bacc
import concourse.bass as bass
import concourse.mybir as mybir
import concourse.tile as tile
from concourse.bass_utils import run_bass_kernel_spmd

dt = mybir.dt

TOKENS, IN_F, OUT_F, GS = 4096, 4096, 11008, 64
G = OUT_F * IN_F // GS            # 704512 quantization groups
J = G // IN_F                     # 172 groups per (g_row, i) plane
NCORES = 8
RPC = GS // NCORES                # 8 g_rows per core
O_C = RPC * J                     # 1376 output cols per core
NK = IN_F // 128                  # 32 contraction blocks
TQ = 256                          # tokens per x-buffer chunk (1 pair)
NQ = TOKENS // TQ                 # 16 pairs
O_SPLITS = ((0, 512), (512, 512), (1024, 352))   # psum o-tiles (1 bank each)

_CACHE = {}


def _build():
    nc = bacc.Bacc("TRN2", target_bir_lowering=False, debug=False,
                   num_devices=NCORES)

    xt_d = nc.dram_tensor("xt", [IN_F, TOKENS], dt.float16,
                          kind="ExternalInput")
    nibf_d = nc.dram_tensor("nibf", [IN_F, O_C], dt.uint8,
                            kind="ExternalInput")
    szt_d = nc.dram_tensor("szt", [IN_F, 2, J], dt.float16,
                           kind="ExternalInput")
    b_d = nc.dram_tensor("bias", [128, O_C], dt.float32,
                         kind="ExternalInput")
    o_d = nc.dram_tensor("out", [TOKENS, O_C], dt.float32,
                         kind="ExternalOutput")

    with ExitStack() as ctx:
        tc = ctx.enter_context(tile.TileContext(nc))
        const = ctx.enter_context(tc.tile_pool(name="const", bufs=1))
        ph1 = ctx.enter_context(tc.tile_pool(name="ph1", bufs=6))
        xpool = ctx.enter_context(tc.tile_pool(name="xpool", bufs=2))
        opool = ctx.enter_context(tc.tile_pool(name="opool", bufs=1))
        pacc = ctx.enter_context(
            tc.tile_pool(name="pacc", bufs=1, space=bass.MemorySpace.PSUM))

        biasrep = const.tile([128, O_C], dt.float32)

        # resident scale/zero: [i-part, k-block, {scale,zero}, j]
        szt = const.tile([128, NK, 2, J], dt.float16)

        # resident transposed dequantized weights: [i-part, k-block, r, j]
        WT = const.tile([128, NK, RPC, J], dt.float16)

        # x-pair prefetch on the SP (sync) DMA stream; issue the first two
        # before any dequant DMA so nothing head-blocks them.
        xbs = {}

        def fetch(q, chunks=1):
            xb = xpool.tile([128, NK, TQ], dt.float16, tag="xb",
                            name=f"xb{q % 2}")
            kc = NK // chunks
            src = xt_d[:, q * TQ:(q + 1) * TQ].rearrange(
                "(k p) t -> p k t", p=128)
            for c in range(chunks):
                nc.sync.dma_start(
                    xb[:, c * kc:(c + 1) * kc], src[:, c * kc:(c + 1) * kc])
            xbs[q] = xb

        fetch(0, chunks=4)
        fetch(1)
        nc.sync.dma_start(biasrep[:], b_d[:])
        ones = const.tile([1, 128], dt.float16)
        nc.vector.memset(ones[:], 1.0)

        # ---- phase 1: dequant (layout already [i, o]; no transposes) ----
        # All dequant DMAs on the ScalarE HWDGE queue (Q1 carries only
        # x/bias/out): szt chunk g issues right before the nib chunk whose
        # k-blocks it covers, nib in [1, 3, 4x7] k-block batches (few,
        # large issues -- a dma_start costs ~0.9us of issuing-queue
        # occupancy; k0 alone so the first sub starts early).  Subs:
        # k%3==2 on GPSIMD (u8 direct, mul deferred 2 k-blocks), k%3==1
        # via ACT u8->fp16 convert + DVE 2x fp16 sub, k%3==0 DVE u8 sub
        # (1x).  Muls d*scale -> WT on DVE (in-place WT muls hit a HW
        # read-write hazard and run ~5x slow).
        nibsrc = nibf_d[:, :].rearrange("(k p) (r j) -> p k r j",
                                        p=128, r=RPC)
        sztsrc = szt_d[:, :].rearrange("(g b p) z j -> p g b z j",
                                       p=128, b=4)
        nibt = {}

        def szt_chunk(g):
            nc.scalar.dma_start(szt[:, 4 * g:4 * (g + 1)], sztsrc[:, g])

        def nib_issue(k0, nk):
            t = ph1.tile([128, nk, RPC, J], dt.uint8, tag=f"nib{nk}",
                         bufs=(3 if nk == 4 else 1))
            nc.scalar.dma_start(t[:], nibsrc[:, k0:k0 + nk])
            for i in range(nk):
                nibt[k0 + i] = t[:, i]

        szt_chunk(0)
        nib_issue(0, 1)
        nib_issue(1, 3)
        szt_chunk(1)
        nib_issue(4, 4)

        pend = []

        def _flush(k, d):
            nc.vector.tensor_mul(
                WT[:, k], d[:],
                szt[:, k, 0].unsqueeze(1).broadcast_to((128, RPC, J)))

        for k in range(NK):
            if k % 4 == 0 and k + 8 < NK:
                szt_chunk((k + 8) // 4)
                nib_issue(k + 8, 4)
            zero_ap = szt[:, k, 1].unsqueeze(1).broadcast_to((128, RPC, J))
            d = ph1.tile([128, RPC, J], dt.float16, tag="d", bufs=5)
            if k >= 2 and (k % 3) == 2:
                nc.gpsimd.tensor_sub(d[:], nibt[k], zero_ap)
                pend.append((k, d))
            else:
                if k % 3 == 1:
                    conv = ph1.tile([128, RPC, J], dt.float16, tag="conv",
                                    bufs=3)
                    nc.scalar.copy(conv[:], nibt[k])
                    nc.vector.tensor_sub(d[:], conv[:], zero_ap)
                else:
                    nc.vector.tensor_sub(d[:], nibt[k], zero_ap)
                _flush(k, d)
            while pend and pend[0][0] <= k - 2:
                _flush(*pend.pop(0))
        while pend:
            _flush(*pend.pop(0))

        # biash only feeds the last pair's PSUM init; emitting it here
        # keeps it from head-blocking the ACT queue's phase-1 work.
        biash = const.tile([1, O_C], dt.float16)
        nc.scalar.copy(biash[:], biasrep[0:1, :])

        # ---- phase 2: stream xT, pair-wise k-outer matmul ----
        # accumulator banks: 512 fp32 = 1 PSUM bank each; rotate 6-of-8
        # per pair so the next pair starts on just-freed banks.
        for q in range(NQ):
            if 2 <= q + 1 < NQ:
                fetch(q + 1)
            xb = xbs.pop(q)
            if q < NQ - 1:
                accs = [[pacc.tile([128, 512], dt.float32,
                                   tag=f"a{(q * 6 + u * 3 + p) % 8}",
                                   name=f"acc{(q * 6 + u * 3 + p) % 8}"
                                   )[:, 0:on]
                         for p, (ob, on) in enumerate(O_SPLITS)]
                        for u in range(2)]
                for k in range(NK):
                    wk = WT[:, k].rearrange("p r j -> p (r j)")
                    for u in range(2):
                        lhsT = xb[:, k, u * 128:(u + 1) * 128]
                        for p, (ob, on) in enumerate(O_SPLITS):
                            nc.tensor.matmul(
                                accs[u][p][:], lhsT, wk[:, ob:ob + on],
                                start=(k == 0), stop=(k == NK - 1))
                for u in range(2):
                    t = q * 2 + u
                    for p, (ob, on) in enumerate(O_SPLITS):
                        ot = opool.tile([128, on], dt.float32,
                                        tag=f"o{u}{p}", name=f"ot{u}{p}")
                        nc.vector.tensor_add(
                            ot[:], accs[u][p][:], biasrep[:, ob:ob + on])
                        nc.sync.dma_start(
                            o_d[t * 128:(t + 1) * 128, ob:ob + on], ot[:])
            else:
                # last pair: o-split-outer / k-inner so each split drains
                # while the next computes; start on the banks pair 14
                # leaves free (a2, a3), then its just-drained ones.
                tags = ["a2", "a3", "a4", "a5", "a6", "a7"]
                for p, (ob, on) in enumerate(O_SPLITS):
                    for u in range(2):
                        acc = pacc.tile([128, 512], dt.float32,
                                        tag=tags[p * 2 + u],
                                        name=f"lacc{p}{u}")[:, 0:on]
                        nc.tensor.matmul(
                            acc[:], ones[0:1, :], biash[0:1, ob:ob + on],
                            start=True, stop=False)
                        for k in range(NK):
                            wk = WT[:, k].rearrange("p r j -> p (r j)")
                            lhsT = xb[:, k, u * 128:(u + 1) * 128]
                            nc.tensor.matmul(
                                acc[:], lhsT, wk[:, ob:ob + on],
                                start=False, stop=(k == NK - 1))
                        t = q * 2 + u
                        ot = opool.tile([128, on], dt.float32,
                                        tag=f"o{u}{p}", name=f"ot{u}{p}")
                        if u == 0:
                            nc.scalar.copy(ot[:], acc[:])
                        else:
                            nc.vector.tensor_copy(ot[:], acc[:])
                        nc.sync.dma_start(
                            o_d[t * 128:(t + 1) * 128, ob:ob + on], ot[:])

    nc.compile()
    return nc


def get_nc():
    if "nc" not in _CACHE:
        _CACHE["nc"] = _build()
    return _CACHE["nc"]


def make_in_maps(x, W_q, scale, zero, bias):
    x = np.ascontiguousarray(x, dtype=np.float32)
    xt = np.ascontiguousarray(x.T).astype(np.float16)
    st = np.asarray(scale, dtype=np.float32).reshape(J, IN_F).T.astype(
        np.float16)
    zt = np.asarray(zero, dtype=np.float32).reshape(J, IN_F).T.astype(
        np.float16)
    szt = np.ascontiguousarray(np.stack([st, zt], axis=1))   # [IN_F, 2, J]
    bias = np.ascontiguousarray(bias, dtype=np.float32)
    Wb = np.asarray(W_q, dtype=np.int32).astype(np.uint8)   # [32, G]
    in_maps = [None] * NCORES
    for cg in range(4):
        slab = Wb[RPC * cg:RPC * (cg + 1)]                  # [8, G]
        for half, c in ((slab >> 4, cg), (slab & 15, cg + 4)):
            nib = np.ascontiguousarray(
                half.reshape(RPC, J, IN_F).transpose(2, 0, 1)
            ).reshape(IN_F, O_C)
            in_maps[c] = {
                "xt": xt,
                "nibf": nib,                                # u8, one nib/byte
                "szt": szt,
                "bias": np.ascontiguousarray(np.broadcast_to(
                    bias[c * O_C:(c + 1) * O_C], (128, O_C))),
            }
    return in_maps


def kernel(x, W_q, scale, zero, bias):
    nc = get_nc()
    in_maps = make_in_maps(x, W_q, scale, zero, bias)
    res = run_bass_kernel_spmd(nc, in_maps, list(range(NCORES)))
    return np.concatenate(
        [res.results[c]["out"] for c in range(NCORES)], axis=1)
